# revision 2
# baseline (speedup 1.0000x reference)
"""TRN2 Bass kernel for ExpertsChooseMaskedExpand MoE routing.

Problem (B=4, T=4096, D=2048, E=8, C=512, O=2048, I=256):
    xr   = x.reshape(B,T,E,I)
    xd   = einsum('btei,btec->beci', xr, dispatch_mask)      # dispatch
    y    = einsum('beci,eoi->beco', xd_homo, w_homo)         # expert mm (+bias)
    out  = einsum('beco,btec->bto', y, combine_array)        # combine

Sharding over 8 cores (core = 2*b + h for batch b, half h):
  - dispatch: T-split — each core contracts its T-half, partial xd
    AllReduced within the (2b, 2b+1) pair.
  - expert mm + combine: O-split — each core produces out[b, :, h-half-of-O]
    for the full T of its batch.

All matmuls run as float32r (TF32-like, full PE rate at free-dim>=256,
~1.7e-4 rel err measured on HW); the rank-1 bias term rides the expert-mm
accumulation group as a bf16 k=1 matmul.
"""
import numpy as np
from contextlib import ExitStack

import concourse.bass as bass
import concourse.tile as tile
from concourse import bacc, mybir
from concourse.bass_utils import run_bass_kernel_spmd

F32 = mybir.dt.float32
F32R = mybir.dt.float32r
BF16 = mybir.dt.bfloat16

B, T, D = 4, 4096, 2048
E, C, O = 8, 512, 2048
I = D // E            # 256
EC = E * C            # 4096
TH = T // 2           # 2048 dispatch tokens per core
OH = O // 2           # 1024 out features per core
NKT = TH // 128       # 16 dispatch t-tiles
NTT = T // 128        # 32 combine t-tiles
NECT = EC // 128      # 32 ec-tiles
N_CORES = 8
REPLICA_PAIRS = [[0, 1], [2, 3], [4, 5], [6, 7]]

_CACHE = {}


def _build():
    nc = bacc.Bacc("TRN2", target_bir_lowering=False, debug=False,
                   num_devices=N_CORES)

    x_s = nc.dram_tensor("x_s", [TH, D], F32R, kind="ExternalInput")
    mask_s = nc.dram_tensor("mask_s", [TH, EC], F32R, kind="ExternalInput")
    combT_s = nc.dram_tensor("combT_s", [NTT, EC, 128], F32R, kind="ExternalInput")
    wT_s = nc.dram_tensor("wT_s", [E, 2, 128, OH], F32R, kind="ExternalInput")
    bias_s = nc.dram_tensor("bias_s", [1, OH], F32, kind="ExternalInput")
    out_s = nc.dram_tensor("out_s", [T, OH], F32, kind="ExternalOutput")

    xd_bounce = nc.dram_tensor("xd_bounce", [E, 2, 128, C], F32R)
    xd_red = nc.dram_tensor("xd_red", [E, 2, 128, C], F32R)

    with ExitStack() as ctx:
        tc = ctx.enter_context(tile.TileContext(nc))
        consts = ctx.enter_context(tc.tile_pool(name="consts", bufs=1))
        p_x = ctx.enter_context(tc.tile_pool(name="p_x", bufs=3))
        p_m = ctx.enter_context(tc.tile_pool(name="p_m", bufs=3))
        p_st = ctx.enter_context(tc.tile_pool(name="p_st", bufs=4))
        p_xd = ctx.enter_context(tc.tile_pool(name="p_xd", bufs=2))
        p_w = ctx.enter_context(tc.tile_pool(name="p_w", bufs=2))
        p_y = ctx.enter_context(tc.tile_pool(name="p_y", bufs=1))
        p_c = ctx.enter_context(tc.tile_pool(name="p_c", bufs=2))
        psum = ctx.enter_context(tc.tile_pool(name="psum", bufs=8, space="PSUM"))

        ones_bf = consts.tile([1, 128], BF16)
        nc.vector.memset(ones_bf[:], 1.0)
        bias_f32 = consts.tile([1, OH], F32)
        nc.sync.dma_start(bias_f32[:], bias_s[:])
        bias_bf = consts.tile([1, OH], BF16)
        nc.vector.tensor_copy(bias_bf[:], bias_f32[:])

        # ---- Phase D: dispatch (xdT[e][i, c] = sum_t x[t, e*I+i] * mask[t, e*C+c])
        # + per-e pairwise AllReduce of the partial xd
        for e in range(E):
            ps = [psum.tile([128, C], F32, tag="ps", name=f"ps_d{e}_{it}")
                  for it in range(2)]
            for kt in range(NKT):
                xt = p_x.tile([128, I], F32R, tag="x")
                nc.sync.dma_start(
                    xt[:], x_s[kt * 128:(kt + 1) * 128, e * I:(e + 1) * I])
                mt = p_m.tile([128, C], F32R, tag="m")
                nc.sync.dma_start(
                    mt[:], mask_s[kt * 128:(kt + 1) * 128, e * C:(e + 1) * C])
                for it in range(2):
                    nc.tensor.matmul(
                        ps[it][:], xt[:, it * 128:(it + 1) * 128], mt[:],
                        start=(kt == 0), stop=(kt == NKT - 1))
            for it in range(2):
                st = p_st.tile([128, C], F32R, tag="st")
                nc.vector.tensor_copy(st[:], ps[it][:])
                nc.sync.dma_start(xd_bounce[e, it], st[:])
            nc.gpsimd.collective_compute(
                "AllReduce", mybir.AluOpType.add,
                replica_groups=REPLICA_PAIRS,
                ins=[xd_bounce[e]], outs=[xd_red[e]])

        # ---- Phase E: expert mm (y[ec, o] = xdT^T @ wT + bias), y stays in SBUF
        y_tiles = []
        for e in range(E):
            xdt = []
            for it in range(2):
                xt = p_xd.tile([128, C], F32R, tag=f"xd{it}", name=f"xd_{e}_{it}")
                nc.sync.dma_start(xt[:], xd_red[e, it])
                xdt.append(xt)
            wt = []
            for it in range(2):
                w = p_w.tile([128, OH], F32R, tag=f"w{it}", name=f"w_{e}_{it}")
                nc.sync.dma_start(w[:], wT_s[e, it])
                wt.append(w)
            for ct in range(4):
                yt = p_y.tile([128, OH], F32R, tag=f"y{e * 4 + ct}")
                y_tiles.append(yt)
                for oc in range(OH // 512):
                    ps = psum.tile([128, 512], F32, tag="ps")
                    for it in range(2):
                        nc.tensor.matmul(
                            ps[:], xdt[it][:, ct * 128:(ct + 1) * 128],
                            wt[it][:, oc * 512:(oc + 1) * 512],
                            start=(it == 0), stop=False)
                    nc.tensor.matmul(
                        ps[:], ones_bf[:], bias_bf[:, oc * 512:(oc + 1) * 512],
                        start=False, stop=True)
                    nc.vector.tensor_copy(yt[:, oc * 512:(oc + 1) * 512], ps[:])

        # ---- Phase C: combine (out[t, o] = sum_ec combT[ec, t] * y[ec, o])
        for tt in range(NTT):
            ctile = p_c.tile([128, NECT, 128], F32R, tag="c")
            nc.sync.dma_start(
                ctile[:], combT_s[tt].rearrange("(a p) t -> p a t", p=128))
            for oc in range(OH // 512):
                ps = psum.tile([128, 512], F32, tag="ps")
                for ec in range(NECT):
                    nc.tensor.matmul(
                        ps[:], ctile[:, ec, :],
                        y_tiles[ec][:, oc * 512:(oc + 1) * 512],
                        start=(ec == 0), stop=(ec == NECT - 1))
                ot = p_st.tile([128, 512], F32, tag="st")
                nc.vector.tensor_copy(ot[:], ps[:])
                nc.sync.dma_start(
                    out_s[tt * 128:(tt + 1) * 128, oc * 512:(oc + 1) * 512],
                    ot[:])

    nc.finalize()
    return nc


def get_nc():
    if "nc" not in _CACHE:
        _CACHE["nc"] = _build()
    return _CACHE["nc"]


def make_in_maps(x, combine_array, dispatch_mask, weight, bias):
    x = np.asarray(x, np.float32)
    combine_array = np.asarray(combine_array, np.float32)
    dispatch_mask = np.asarray(dispatch_mask, np.float32)
    weight = np.asarray(weight, np.float32)
    bias = np.asarray(bias, np.float32)

    in_maps = []
    combT_by_b = {}
    for core in range(N_CORES):
        b, h = divmod(core, 2)
        if b not in combT_by_b:
            comb_b = combine_array[b].reshape(T, EC)
            combT_by_b[b] = np.ascontiguousarray(
                comb_b.reshape(NTT, 128, EC).transpose(0, 2, 1))
        wT = np.ascontiguousarray(
            weight[:, h * OH:(h + 1) * OH, :].transpose(0, 2, 1)
        ).reshape(E, 2, 128, OH)
        in_maps.append({
            "x_s": np.ascontiguousarray(x[b, h * TH:(h + 1) * TH, :]),
            "mask_s": np.ascontiguousarray(
                dispatch_mask[b, h * TH:(h + 1) * TH].reshape(TH, EC)),
            "combT_s": combT_by_b[b],
            "wT_s": wT,
            "bias_s": np.ascontiguousarray(bias[h * OH:(h + 1) * OH]).reshape(1, OH),
        })
    return in_maps


def assemble(results):
    out = np.empty((B, T, O), np.float32)
    for core in range(N_CORES):
        b, h = divmod(core, 2)
        out[b, :, h * OH:(h + 1) * OH] = results[core]["out_s"]
    return out


def kernel(x, combine_array, dispatch_mask, weight, bias):
    nc = get_nc()
    in_maps = make_in_maps(x, combine_array, dispatch_mask, weight, bias)
    res = run_bass_kernel_spmd(nc, in_maps, list(range(N_CORES)))
    return assemble(res.results)


# revision 6
# speedup vs baseline: 2.0132x; 2.0132x over previous
"""TRN2 Bass kernel for ExpertsChooseMaskedExpand MoE routing.

Problem (B=4, T=4096, D=2048, E=8, C=512, O=2048, I=256):
    xr   = x.reshape(B,T,E,I)
    xd   = einsum('btei,btec->beci', xr, dispatch_mask)      # dispatch
    y    = einsum('beci,eoi->beco', xd_homo, w_homo)         # expert mm (+bias)
    out  = einsum('beco,btec->bto', y, combine_array)        # combine

Sharding over 8 cores (core = 2*b + h for batch b, half h):
  - dispatch: T-split — each core contracts its T-half; the partial xd is
    AllReduced (fp32) within the (2b, 2b+1) pair, in two 4-expert groups so
    the second collective overlaps the first group's expert matmuls.
  - expert mm + combine: O-split — each core produces out[b, :, h-half-of-O]
    for the full T of its batch, with y (all experts x its O-half) resident
    in SBUF.

Matmul datapath is bf16 (measured 217 ns per 128x128x512 MM on HW = PE
peak; fp32r measured 2x slower on this toolchain). PSUM accumulation is
fp32; the xd pair-reduction stays fp32. The rank-1 bias term rides the
expert-mm accumulation group as a k=1 bf16 matmul.
"""
import numpy as np
import ml_dtypes
from contextlib import ExitStack

import concourse.bass as bass
import concourse.tile as tile
from concourse import bacc, mybir
from concourse.bass_utils import run_bass_kernel_spmd

F32 = mybir.dt.float32
BF16 = mybir.dt.bfloat16
NP_BF16 = ml_dtypes.bfloat16

B, T, D = 4, 4096, 2048
E, C, O = 8, 512, 2048
I = D // E            # 256
EC = E * C            # 4096
TH = T // 2           # 2048 dispatch tokens per core
OH = O // 2           # 1024 out features per core
NKT = TH // 128       # 16 dispatch t-tiles
NTP = T // 256        # 16 combine t-superblocks (256 tokens each)
NECT = EC // 128      # 32 ec-tiles
N_CORES = 8
REPLICA_PAIRS = [[0, 1], [2, 3], [4, 5], [6, 7]]

_CACHE = {}


def _build(repeat=1, skip_ar=False):
    nc = bacc.Bacc("TRN2", target_bir_lowering=False, debug=False,
                   num_devices=N_CORES)

    x_s = nc.dram_tensor("x_s", [TH, D], BF16, kind="ExternalInput")
    mask_s = nc.dram_tensor("mask_s", [TH, EC], BF16, kind="ExternalInput")
    combT_s = nc.dram_tensor("combT_s", [NTP, EC, 256], BF16, kind="ExternalInput")
    wT_s = nc.dram_tensor("wT_s", [E, 2, 128, OH], BF16, kind="ExternalInput")
    bias_s = nc.dram_tensor("bias_s", [1, OH], BF16, kind="ExternalInput")
    out_s = nc.dram_tensor("out_s", [T, OH], F32, kind="ExternalOutput")

    xd_bounce = nc.dram_tensor("xd_bounce", [E, 2, 128, C], F32)
    xd_red = nc.dram_tensor("xd_red", [E, 2, 128, C], F32)

    with ExitStack() as ctx:
        tc = ctx.enter_context(tile.TileContext(nc))
        consts = ctx.enter_context(tc.tile_pool(name="consts", bufs=1))
        p_x = ctx.enter_context(tc.tile_pool(name="p_x", bufs=3))
        p_m = ctx.enter_context(tc.tile_pool(name="p_m", bufs=3))
        p_st = ctx.enter_context(tc.tile_pool(name="p_st", bufs=4))
        p_xd = ctx.enter_context(tc.tile_pool(name="p_xd", bufs=2))
        p_w = ctx.enter_context(tc.tile_pool(name="p_w", bufs=2))
        p_y = ctx.enter_context(tc.tile_pool(name="p_y", bufs=1))
        p_c = ctx.enter_context(tc.tile_pool(name="p_c", bufs=2))
        psum = ctx.enter_context(tc.tile_pool(name="psum", bufs=8, space="PSUM"))

        def emit_body():
            _emit(nc, tc, consts, p_x, p_m, p_st, p_xd, p_w, p_y, p_c, psum,
                  x_s, mask_s, combT_s, wT_s, bias_s, out_s, xd_bounce, xd_red,
                  skip_ar)

        if repeat > 1:
            with tc.For_i(0, repeat, 1):
                emit_body()
        else:
            emit_body()

    nc.finalize()
    return nc


def _emit(nc, tc, consts, p_x, p_m, p_st, p_xd, p_w, p_y, p_c, psum,
          x_s, mask_s, combT_s, wT_s, bias_s, out_s, xd_bounce, xd_red,
          skip_ar=False):
    ones_bf = consts.tile([1, 128], BF16)
    nc.vector.memset(ones_bf[:], 1.0)
    bias_bf = consts.tile([1, OH], BF16)
    nc.sync.dma_start(bias_bf[:], bias_s[:])

    # ---- Phase D: dispatch (xdT[e][i, c] = sum_t x[t, e*I+i] * mask[t, e*C+c])
    for e in range(E):
        ps = [psum.tile([128, C], F32, tag="ps", name=f"ps_d{e}_{it}")
              for it in range(2)]
        for kt in range(NKT):
            xt = p_x.tile([128, I], BF16, tag="x", name=f"x_{e}_{kt}")
            nc.sync.dma_start(
                xt[:], x_s[kt * 128:(kt + 1) * 128, e * I:(e + 1) * I])
            mt = p_m.tile([128, C], BF16, tag="m", name=f"m_{e}_{kt}")
            nc.sync.dma_start(
                mt[:], mask_s[kt * 128:(kt + 1) * 128, e * C:(e + 1) * C])
            for it in range(2):
                nc.tensor.matmul(
                    ps[it][:], xt[:, it * 128:(it + 1) * 128], mt[:],
                    start=(kt == 0), stop=(kt == NKT - 1))
        for it in range(2):
            st = p_st.tile([128, C], F32, tag="st", name=f"st_d{e}_{it}")
            nc.vector.tensor_copy(st[:], ps[it][:])
            nc.sync.dma_start(xd_bounce[e, it], st[:])
        # fp32 pairwise AllReduce of partial xd, in two 4-expert groups
        if e in (3, 7) and not skip_ar:
            g = e - 3
            nc.gpsimd.collective_compute(
                "AllReduce", mybir.AluOpType.add,
                replica_groups=REPLICA_PAIRS,
                ins=[xd_bounce[g:g + 4]], outs=[xd_red[g:g + 4]])

    # ---- Phase E: expert mm (y[ec, o] = xdT^T @ wT + bias), y resident bf16
    y_tiles = []
    for e in range(E):
        xdt = []
        for it in range(2):
            xf = p_xd.tile([128, C], F32, tag=f"xdf{it}", name=f"xdf_{e}_{it}")
            nc.sync.dma_start(
                xf[:], (xd_bounce if skip_ar else xd_red)[e, it])
            xb = p_xd.tile([128, C], BF16, tag=f"xdb{it}", name=f"xdb_{e}_{it}")
            nc.vector.tensor_copy(xb[:], xf[:])
            xdt.append(xb)
        wt = []
        for it in range(2):
            w = p_w.tile([128, OH], BF16, tag=f"w{it}", name=f"w_{e}_{it}")
            nc.sync.dma_start(w[:], wT_s[e, it])
            wt.append(w)
        for ct in range(4):
            yt = p_y.tile([128, OH], BF16, tag=f"y{e * 4 + ct}",
                          name=f"y_{e}_{ct}")
            y_tiles.append(yt)
            for oc in range(OH // 512):
                ps = psum.tile([128, 512], F32, tag="ps",
                               name=f"ps_e{e}_{ct}_{oc}")
                for it in range(2):
                    nc.tensor.matmul(
                        ps[:], xdt[it][:, ct * 128:(ct + 1) * 128],
                        wt[it][:, oc * 512:(oc + 1) * 512],
                        start=(it == 0), stop=False)
                nc.tensor.matmul(
                    ps[:], ones_bf[:], bias_bf[:, oc * 512:(oc + 1) * 512],
                    start=False, stop=True)
                nc.vector.tensor_copy(yt[:, oc * 512:(oc + 1) * 512], ps[:])

    # ---- Phase C: combine (out[t, o] = sum_ec combT[ec, t] * y[ec, o])
    for tp in range(NTP):
        ctile = p_c.tile([128, NECT, 256], BF16, tag="c", name=f"c_{tp}")
        nc.sync.dma_start(
            ctile[:], combT_s[tp].rearrange("(a p) t -> p a t", p=128))
        for ts in range(2):
            tt = tp * 2 + ts
            for oc in range(OH // 512):
                ps = psum.tile([128, 512], F32, tag="ps",
                               name=f"ps_c{tt}_{oc}")
                for ec in range(NECT):
                    nc.tensor.matmul(
                        ps[:], ctile[:, ec, ts * 128:(ts + 1) * 128],
                        y_tiles[ec][:, oc * 512:(oc + 1) * 512],
                        start=(ec == 0), stop=(ec == NECT - 1))
                ot = p_st.tile([128, 512], F32, tag="st", name=f"ot_{tt}_{oc}")
                nc.vector.tensor_copy(ot[:], ps[:])
                nc.sync.dma_start(
                    out_s[tt * 128:(tt + 1) * 128, oc * 512:(oc + 1) * 512],
                    ot[:])


def get_nc():
    if "nc" not in _CACHE:
        _CACHE["nc"] = _build()
    return _CACHE["nc"]


def make_in_maps(x, combine_array, dispatch_mask, weight, bias):
    x = np.asarray(x, np.float32)
    combine_array = np.asarray(combine_array, np.float32)
    dispatch_mask = np.asarray(dispatch_mask, np.float32)
    weight = np.asarray(weight, np.float32)
    bias = np.asarray(bias, np.float32)

    in_maps = []
    combT_by_b = {}
    for core in range(N_CORES):
        b, h = divmod(core, 2)
        if b not in combT_by_b:
            comb_b = combine_array[b].reshape(T, EC).astype(NP_BF16)
            combT_by_b[b] = np.ascontiguousarray(
                comb_b.reshape(NTP, 256, EC).transpose(0, 2, 1))
        wT = np.ascontiguousarray(
            weight[:, h * OH:(h + 1) * OH, :].transpose(0, 2, 1).astype(NP_BF16)
        ).reshape(E, 2, 128, OH)
        in_maps.append({
            "x_s": np.ascontiguousarray(
                x[b, h * TH:(h + 1) * TH, :]).astype(NP_BF16),
            "mask_s": np.ascontiguousarray(
                dispatch_mask[b, h * TH:(h + 1) * TH].reshape(TH, EC)
            ).astype(NP_BF16),
            "combT_s": combT_by_b[b],
            "wT_s": wT,
            "bias_s": bias[h * OH:(h + 1) * OH].reshape(1, OH).astype(NP_BF16),
        })
    return in_maps


def assemble(results):
    out = np.empty((B, T, O), np.float32)
    for core in range(N_CORES):
        b, h = divmod(core, 2)
        out[b, :, h * OH:(h + 1) * OH] = results[core]["out_s"]
    return out


def kernel(x, combine_array, dispatch_mask, weight, bias):
    nc = get_nc()
    in_maps = make_in_maps(x, combine_array, dispatch_mask, weight, bias)
    res = run_bass_kernel_spmd(nc, in_maps, list(range(N_CORES)))
    return assemble(res.results)


# revision 17
# speedup vs baseline: 122.1014x; 60.6493x over previous
"""TRN2 Bass kernel for ExpertsChooseMaskedExpand MoE routing.

Problem (B=4, T=4096, D=2048, E=8, C=512, O=2048, I=256):
    xr   = x.reshape(B,T,E,I)
    xd   = einsum('btei,btec->beci', xr, dispatch_mask)      # dispatch
    y    = einsum('beci,eoi->beco', xd_homo, w_homo)         # expert mm (+bias)
    out  = einsum('beco,btec->bto', y, combine_array)        # combine

Sharding over 8 cores (core = 2*b + h for batch b, half h):
  - dispatch: T-split — each core contracts its T-half; the partial xd is
    AllReduced (fp32) within the (2b, 2b+1) pair, in two 4-expert groups so
    the second collective overlaps the first group's expert matmuls.
  - expert mm + combine: O-split — each core produces out[b, :, h-half-of-O]
    for the full T of its batch, with y (all experts x its O-half) resident
    in SBUF.

Matmul datapath is bf16 (measured 217 ns per 128x128x512 MM on HW = PE
peak; fp32r measured 2x slower on this toolchain). PSUM accumulation is
fp32; the xd pair-reduction stays fp32. The rank-1 bias term rides the
expert-mm accumulation group as a k=1 bf16 matmul.
"""
import numpy as np
import ml_dtypes
from contextlib import ExitStack

import concourse.bass as bass
import concourse.tile as tile
from concourse.tile_rust import add_dep_helper
from concourse import bacc, mybir
from concourse.bass_utils import run_bass_kernel_spmd

F32 = mybir.dt.float32
BF16 = mybir.dt.bfloat16
NP_BF16 = ml_dtypes.bfloat16

B, T, D = 4, 4096, 2048
E, C, O = 8, 512, 2048
I = D // E            # 256
EC = E * C            # 4096
TH = T // 2           # 2048 dispatch tokens per core
OH = O // 2           # 1024 out features per core
NKT = TH // 128       # 16 dispatch t-tiles
NTP = T // 256        # 16 combine t-superblocks (256 tokens each)
NECT = EC // 128      # 32 ec-tiles
N_CORES = 8
REPLICA_PAIRS = [[0, 1], [2, 3], [4, 5], [6, 7]]

_CACHE = {}


def _build(repeat=1, skip_ar=False, phases="DEC", no_bias=False):
    nc = bacc.Bacc("TRN2", target_bir_lowering=False, debug=False,
                   num_devices=N_CORES)

    x_s = nc.dram_tensor("x_s", [TH, D], BF16, kind="ExternalInput")
    mask_s = nc.dram_tensor("mask_s", [TH, EC], BF16, kind="ExternalInput")
    combT_s = nc.dram_tensor("combT_s", [NTP, EC, 256], BF16, kind="ExternalInput")
    wT_s = nc.dram_tensor("wT_s", [E, 2, 128, OH], BF16, kind="ExternalInput")
    bias_s = nc.dram_tensor("bias_s", [1, OH], F32, kind="ExternalInput")
    out_s = nc.dram_tensor("out_s", [T, OH], F32, kind="ExternalOutput")

    xd_bounce = [nc.dram_tensor(f"xd_bounce{g}", [4, 2, 128, C], F32)
                 for g in range(2)]
    xd_red = [nc.dram_tensor(f"xd_red{g}", [4, 2, 128, C], F32)
              for g in range(2)]

    with ExitStack() as ctx:
        tc = ctx.enter_context(tile.TileContext(nc))
        consts = ctx.enter_context(tc.tile_pool(name="consts", bufs=1))
        p_x = ctx.enter_context(tc.tile_pool(name="p_x", bufs=1))
        p_m = ctx.enter_context(tc.tile_pool(name="p_m", bufs=13))
        p_st = ctx.enter_context(tc.tile_pool(name="p_st", bufs=4))
        p_xd = ctx.enter_context(tc.tile_pool(name="p_xd", bufs=2))
        p_w = ctx.enter_context(tc.tile_pool(name="p_w", bufs=2))
        p_y = ctx.enter_context(tc.tile_pool(name="p_y", bufs=1))
        p_c = ctx.enter_context(tc.tile_pool(name="p_c", bufs=2))
        psum = ctx.enter_context(tc.tile_pool(name="psum", bufs=8, space="PSUM"))
        # per-phase PSUM tags so a stalled phase can't starve another's banks
        psum_tags = {"D": ("psd", 3), "E": ("pse", 2), "C": ("psc", 3)}

        def emit_body():
            _emit(nc, tc, consts, p_x, p_m, p_st, p_xd, p_w, p_y, p_c, psum,
                  x_s, mask_s, combT_s, wT_s, bias_s, out_s, xd_bounce, xd_red,
                  skip_ar, phases, no_bias, psum_tags)

        if repeat > 1:
            with tc.For_i(0, repeat, 1):
                emit_body()
        else:
            emit_body()

    nc.finalize()
    return nc


def _emit(nc, tc, consts, p_x, p_m, p_st, p_xd, p_w, p_y, p_c, psum,
          x_s, mask_s, combT_s, wT_s, bias_s, out_s, xd_bounce, xd_red,
          skip_ar=False, phases="DEC", no_bias=False, psum_tags=None):
    # bias replicated across partitions once; folded into the psum->y copy
    bias_rep = consts.tile([128, OH], F32)
    nc.sync.dma_start(bias_rep[:], bias_s[:].partition_broadcast(128))

    # ---- Phase D: dispatch (xdT[e][i, c] = sum_t x[t, e*I+i] * mask[t, e*C+c])
    # x is preloaded resident (16 big DMAs with 4KB lines beat 256 small ones)
    run_d = "D" in phases
    run_e = "E" in phases
    run_c = "C" in phases
    xres = []
    m0 = []
    for kt in range(NKT if run_d else 0):
        xr = p_x.tile([128, D], BF16, tag=f"xres{kt}", name=f"xres_{kt}")
        nc.sync.dma_start(xr[:], x_s[kt * 128:(kt + 1) * 128, :])
        xres.append(xr)
        # interleave e=0 mask loads so the first matmuls aren't queued
        # behind the whole x preload
        mt = p_m.tile([128, C], BF16, tag="m", name=f"m_0_{kt}")
        nc.sync.dma_start(mt[:], mask_s[kt * 128:(kt + 1) * 128, 0:C])
        m0.append(mt)
    for e in range(E if run_d else 0):
        tg, bf = psum_tags["D"]
        ps = [psum.tile([128, C], F32, tag=tg, bufs=bf, name=f"ps_d{e}_{it}")
              for it in range(2)]
        for kt in range(NKT):
            if e == 0:
                mt = m0[kt]
            else:
                mt = p_m.tile([128, C], BF16, tag="m", name=f"m_{e}_{kt}")
                nc.sync.dma_start(
                    mt[:], mask_s[kt * 128:(kt + 1) * 128, e * C:(e + 1) * C])
            for it in range(2):
                nc.tensor.matmul(
                    ps[it][:],
                    xres[kt][:, e * I + it * 128:e * I + (it + 1) * 128],
                    mt[:], start=(kt == 0), stop=(kt == NKT - 1))
        for it in range(2):
            st = p_st.tile([128, C], F32, tag="st", name=f"st_d{e}_{it}")
            nc.vector.tensor_copy(st[:], ps[it][:])
            nc.sync.dma_start(xd_bounce[e // 4][e % 4, it], st[:])
        # fp32 pairwise AllReduce of partial xd, in two 4-expert groups
        if e in (3, 7) and not skip_ar:
            g = e // 4
            nc.gpsimd.collective_compute(
                "AllReduce", mybir.AluOpType.add,
                replica_groups=REPLICA_PAIRS,
                ins=[xd_bounce[g][:]], outs=[xd_red[g][:]])

    # ---- Phase E: expert mm (y[ec, o] = xdT^T @ wT + bias), y resident bf16
    y_tiles = []
    last_ycopy = [None]
    for e in range(E if run_e else 0):
        xdt = []
        for it in range(2):
            xf = p_xd.tile([128, C], F32, tag=f"xdf{it}", name=f"xdf_{e}_{it}")
            src = (xd_bounce if skip_ar else xd_red)[e // 4]
            nc.sync.dma_start(xf[:], src[e % 4, it])
            xb = p_xd.tile([128, C], BF16, tag=f"xdb{it}", name=f"xdb_{e}_{it}")
            nc.vector.tensor_copy(xb[:], xf[:])
            xdt.append(xb)
        wt = []
        for it in range(2):
            w = p_w.tile([128, OH], BF16, tag=f"w{it}", name=f"w_{e}_{it}")
            nc.sync.dma_start(w[:], wT_s[e, it])
            wt.append(w)
        for ct in range(4):
            yt = p_y.tile([128, OH], BF16, tag=f"y{e * 4 + ct}",
                          name=f"y_{e}_{ct}")
            y_tiles.append(yt)
            for oc in range(OH // 512):
                tg, bf = psum_tags["E"]
                ps = psum.tile([128, 512], F32, tag=tg, bufs=bf,
                               name=f"ps_e{e}_{ct}_{oc}")
                for it in range(2):
                    nc.tensor.matmul(
                        ps[:], xdt[it][:, ct * 128:(ct + 1) * 128],
                        wt[it][:, oc * 512:(oc + 1) * 512],
                        start=(it == 0), stop=(it == 1))
                if no_bias:
                    ycopy = nc.vector.tensor_copy(
                        yt[:, oc * 512:(oc + 1) * 512], ps[:])
                else:
                    ycopy = nc.vector.tensor_add(
                        yt[:, oc * 512:(oc + 1) * 512], ps[:],
                        bias_rep[:, oc * 512:(oc + 1) * 512])
                last_ycopy[0] = ycopy.ins

    # ---- Phase C: combine (out[t, o] = sum_ec combT[ec, t] * y[ec, o])
    if run_c and not run_e:
        for i in range(32):
            yt = p_y.tile([128, OH], BF16, tag=f"y{i}", name=f"ym_{i}")
            nc.vector.memset(yt[:], 0.25)
            y_tiles.append(yt)
    for tp in range(NTP if run_c else 0):
        ctile = p_c.tile([128, NECT, 256], BF16, tag="c", name=f"c_{tp}")
        nc.sync.dma_start(
            ctile[:], combT_s[tp].rearrange("(a p) t -> p a t", p=128))
        for ts in range(2):
            tt = tp * 2 + ts
            for oc in range(OH // 512):
                tg, bf = psum_tags["C"]
                ps = psum.tile([128, 512], F32, tag=tg, bufs=bf,
                               name=f"ps_c{tt}_{oc}")
                for ec in range(NECT):
                    mm = nc.tensor.matmul(
                        ps[:], ctile[:, ec, ts * 128:(ts + 1) * 128],
                        y_tiles[ec][:, oc * 512:(oc + 1) * 512],
                        start=(ec == 0), stop=(ec == NECT - 1))

                ot = p_st.tile([128, 512], F32, tag="st", name=f"ot_{tt}_{oc}")
                nc.vector.tensor_copy(ot[:], ps[:])
                nc.sync.dma_start(
                    out_s[tt * 128:(tt + 1) * 128, oc * 512:(oc + 1) * 512],
                    ot[:])
    if run_e and not run_c:
        ot = p_st.tile([128, 512], F32, tag="st", name="ot_keep")
        nc.vector.tensor_copy(ot[:], y_tiles[-1][:, 0:512])
        nc.sync.dma_start(out_s[0:128, 0:512], ot[:])
    if run_d and not run_e and not run_c:
        xf = p_xd.tile([128, C], F32, tag="xdf0", name="xf_keep")
        nc.sync.dma_start(xf[:], xd_bounce[1][3, 1])
        ot = p_st.tile([128, 512], F32, tag="st", name="ot_keep")
        nc.vector.tensor_copy(ot[:], xf[:])
        nc.sync.dma_start(out_s[0:128, 0:512], ot[:])


def get_nc():
    if "nc" not in _CACHE:
        _CACHE["nc"] = _build()
    return _CACHE["nc"]


def make_in_maps(x, combine_array, dispatch_mask, weight, bias):
    x = np.asarray(x, np.float32)
    combine_array = np.asarray(combine_array, np.float32)
    dispatch_mask = np.asarray(dispatch_mask, np.float32)
    weight = np.asarray(weight, np.float32)
    bias = np.asarray(bias, np.float32)

    in_maps = []
    combT_by_b = {}
    for core in range(N_CORES):
        b, h = divmod(core, 2)
        if b not in combT_by_b:
            comb_b = combine_array[b].reshape(T, EC).astype(NP_BF16)
            combT_by_b[b] = np.ascontiguousarray(
                comb_b.reshape(NTP, 256, EC).transpose(0, 2, 1))
        wT = np.ascontiguousarray(
            weight[:, h * OH:(h + 1) * OH, :].transpose(0, 2, 1).astype(NP_BF16)
        ).reshape(E, 2, 128, OH)
        in_maps.append({
            "x_s": np.ascontiguousarray(
                x[b, h * TH:(h + 1) * TH, :]).astype(NP_BF16),
            "mask_s": np.ascontiguousarray(
                dispatch_mask[b, h * TH:(h + 1) * TH].reshape(TH, EC)
            ).astype(NP_BF16),
            "combT_s": combT_by_b[b],
            "wT_s": wT,
            "bias_s": np.ascontiguousarray(bias[h * OH:(h + 1) * OH]).reshape(1, OH),
        })
    return in_maps


def assemble(results):
    out = np.empty((B, T, O), np.float32)
    for core in range(N_CORES):
        b, h = divmod(core, 2)
        out[b, :, h * OH:(h + 1) * OH] = results[core]["out_s"]
    return out


def kernel(x, combine_array, dispatch_mask, weight, bias):
    nc = get_nc()
    in_maps = make_in_maps(x, combine_array, dispatch_mask, weight, bias)
    res = run_bass_kernel_spmd(nc, in_maps, list(range(N_CORES)))
    return assemble(res.results)
